# revision 8
# baseline (speedup 1.0000x reference)
"""Multi-head attention (B=4, S=2048, D=1024, H=16, d_k=64) on 8 NeuronCores.

Sharding: core c handles batch b=c//2 and head-group g=c%2 (8 heads, 512
features). Each core computes Q/K/V projections for its head group,
attention, and a partial output projection (row-split W_o). Host sums the
two partial outputs per batch.

V3 schedule: software-pipelined phases so the ACT (exp) and PE streams
stay busy end to end.
- K^T projection first (head), then per sequence-group sg (512 queries):
  Q^T projection for sg, QK scores, exp, PV — with V projection groups
  interleaved into sg0's PE stream and out-projection of sg-1 interleaved
  into sg's stream.
- Scores in "scores^T" layout [sk, sq]; softmax sums via a ones-column
  appended to V (row 64 of the PV PSUM accumulator). No max-subtraction
  (|score/8| <= ~7, exp is safe in fp32).
- PV runs as two 16-matmul accumulation chains per (head-pair, sg):
  all sk for head h2=0, then h2=1, so only 2 PSUM accumulators are live
  and chain h2=1 covers the normalization latency of h2=0.
- PSUM budget: QK tiles [128,1024]x2 (4 banks) + PV [65,512]x2 (2) +
  a shared [128,512]x2 ring for K/Q/V-proj and out-proj (2) = 8 banks.
- Out projection accumulates in PSUM and DMAs straight to DRAM (no DVE
  copy); normalization reciprocal+broadcast run on DVE+Pool.
"""

import sys

sys.path.insert(0, "/opt/trn_rl_repo")

import numpy as np
import ml_dtypes

BF = ml_dtypes.bfloat16

S = 2048          # sequence length
D = 1024          # model dim
F = 512           # features per core (8 heads x 64)
HPC = 8           # heads per core
DK = 64           # head dim
P = 128           # partitions
NCORES = 8
KC = D // P       # 8 contraction chunks for projections
ST = S // P       # 16 sequence tiles of 128
SG = S // 512     # 4 sequence groups of 512
FC = F // P       # 4 feature chunks of 128


def _build_program(reps=1):
    import concourse.bass as bass
    import concourse.mybir as mybir
    import concourse.tile as tile
    from concourse import bacc

    dt = mybir.dt
    f32 = dt.float32
    bf16 = dt.bfloat16
    EXP = mybir.ActivationFunctionType.Exp

    nc = bacc.Bacc("TRN2", target_bir_lowering=False, debug=False,
                   num_devices=NCORES)

    xq_d = nc.declare_dram_parameter("xq", [D, S], bf16, isOutput=False)
    xk_d = nc.declare_dram_parameter("xk", [D, S], bf16, isOutput=False)
    xv_d = nc.declare_dram_parameter("xv", [D, S], bf16, isOutput=False)
    wq_d = nc.declare_dram_parameter("wq", [D, F], bf16, isOutput=False)
    wk_d = nc.declare_dram_parameter("wk", [D, F], bf16, isOutput=False)
    wv_d = nc.declare_dram_parameter("wv", [D, F], bf16, isOutput=False)
    wo_d = nc.declare_dram_parameter("wo", [F, D], bf16, isOutput=False)
    out_d = nc.declare_dram_parameter("out", [S, D], f32, isOutput=True)

    xq_t = xq_d.ap().rearrange("(c p) s -> c p s", p=P)
    xk_t = xk_d.ap().rearrange("(c p) s -> c p s", p=P)
    xv_t = xv_d.ap().rearrange("(c p) s -> c p s", p=P)
    wq_t = wq_d.ap().rearrange("(c p) f -> c p f", p=P)
    wk_t = wk_d.ap().rearrange("(c p) f -> c p f", p=P)
    wv_t = wv_d.ap().rearrange("(c p) f -> c p f", p=P)
    wo_t = wo_d.ap().rearrange("(c p) o -> c p o", p=P)
    out_t = out_d.ap().rearrange("(t p) o -> t p o", p=P)

    with tile.TileContext(nc) as tc:
      # Pools are shared across reps: rep r+1 reuses the same SBUF/PSUM
      # rings (tile WAR deps pipeline the reps), so multi-rep NEFFs run at
      # steady-state engine throughput instead of serializing whole reps.
      with (
          tc.tile_pool(name="wpool", bufs=1) as wpool,
          tc.tile_pool(name="xkv", bufs=8) as xkv,
          tc.tile_pool(name="xqp", bufs=8) as xqp,
          tc.tile_pool(name="xvq", bufs=16) as xvq,
          tc.tile_pool(name="qkpool", bufs=1) as qkpool,
          tc.tile_pool(name="vpool", bufs=1) as vpool,
          tc.tile_pool(name="apool", bufs=8) as apool,
          tc.tile_pool(name="cpool", bufs=1) as cpool,
          tc.tile_pool(name="spool", bufs=3) as spool,
          tc.tile_pool(name="mmps", bufs=2, space="PSUM") as mmps,
          tc.tile_pool(name="pvps", bufs=2, space="PSUM") as pvps,
          tc.tile_pool(name="vops", bufs=2, space="PSUM") as vops,
      ):
        for rep in range(reps):
            # ---- weights + staged x (issue order controls DMA queue) ----
            w_sb = {}

            def load_w(nm, src):
                for c in range(KC):
                    t = wpool.tile([P, F], bf16, tag=f"{nm}{c}", name=f"{nm}{c}")
                    nc.sync.dma_start(t[:], src[c])
                    w_sb[nm, c] = t

            load_w("wk", wk_t)
            xk_sb = []
            for c in range(KC):
                xt = xkv.tile([P, S], bf16, tag="xkv", name=f"xk{c}")
                nc.sync.dma_start(xt[:], xk_t[c])
                xk_sb.append(xt)
            load_w("wq", wq_t)


            # per-sg xq DMA rings
            xq_group = {}

            def issue_xq_dma(sg):
                tiles = []
                for c in range(KC):
                    xt = xqp.tile([P, 512], bf16, tag="xq", name=f"xq{sg}_{c}")
                    nc.sync.dma_start(
                        xt[:], xq_t[c][:, sg * 512:(sg + 1) * 512])
                    tiles.append(xt)
                xq_group[sg] = tiles

            issue_xq_dma(0)
            load_w("wv", wv_t)
            wo_sb = []
            for c in range(FC):
                t = wpool.tile([P, D], bf16, tag=f"wo{c}", name=f"wo{c}")
                nc.sync.dma_start(t[:], wo_t[c])
                wo_sb.append(t)
            # xv: quarter-column tiles [128,512] per (quarter, c), own ring
            xv_sb = {}
            for q in range(4):
                for c in range(KC):
                    xt = xvq.tile([P, 512], bf16, tag="xv", name=f"xv{q}_{c}")
                    nc.sync.dma_start(
                        xt[:], xv_t[c][:, q * 512:(q + 1) * 512])
                    xv_sb[q, c] = xt
            issue_xq_dma(1)

            qt_sb = [qkpool.tile([P, S], bf16, tag=f"qt{i}", name=f"qt{i}")
                     for i in range(FC)]
            kt_sb = [qkpool.tile([P, S], bf16, tag=f"kt{i}", name=f"kt{i}")
                     for i in range(FC)]
            ctx_sb = [cpool.tile([P, S], bf16, tag=f"ctx{i}", name=f"ctx{i}")
                      for i in range(FC)]
            # v_sb[t]: [128, 8 heads, 64 features + ones column]
            v_sb = [vpool.tile([P, HPC, DK + 1], bf16, tag=f"v{t}",
                               name=f"v{t}")
                    for t in range(ST)]
            for t in range(ST):
                nc.gpsimd.memset(v_sb[t][:], 1.0)

            def k_proj_fc(fc):
                # all 4 sg column-blocks of kt_sb[fc]
                for sg in range(SG):
                    ps = vops.tile([P, 512], f32, tag="vo", name="ps_k")
                    for c in range(KC):
                        nc.tensor.matmul(
                            ps[:],
                            w_sb["wk", c][:, fc * P:(fc + 1) * P],
                            xk_sb[c][:, sg * 512:(sg + 1) * 512],
                            start=(c == 0), stop=(c == KC - 1),
                        )
                    nc.vector.tensor_copy(
                        kt_sb[fc][:, sg * 512:(sg + 1) * 512], ps[:])

            def q_proj_group(sg, fc):
                # Q^T projection for one (sg, fc): xq arrives via a
                # per-(sg,c) ring of [128,512] tiles.
                ps = vops.tile([P, 512], f32, tag="vo", name="ps_q")
                for c in range(KC):
                    nc.tensor.matmul(
                        ps[:],
                        w_sb["wq", c][:, fc * P:(fc + 1) * P],
                        xq_group[sg][c][:, :],
                        start=(c == 0), stop=(c == KC - 1),
                    )
                nc.vector.tensor_copy(
                    qt_sb[fc][:, sg * 512:(sg + 1) * 512], ps[:])

            def v_proj_group(t):
                q, r = t // 4, t % 4
                ps = vops.tile([P, 512], f32, tag="vo", name="ps_v")
                for c in range(KC):
                    nc.tensor.matmul(
                        ps[:],
                        xv_sb[q, c][:, r * P:(r + 1) * P],
                        w_sb["wv", c][:],
                        start=(c == 0), stop=(c == KC - 1),
                    )
                nc.vector.tensor_copy(
                    v_sb[t][:, :, 0:DK],
                    ps.rearrange("p (h d) -> p h d", h=HPC))

            def out_proj_group(t, og):
                ps = vops.tile([P, 512], f32, tag="vo", name="ps_out")
                for fc in range(FC):
                    nc.tensor.matmul(
                        ps[:],
                        ctx_sb[fc][:, t * P:(t + 1) * P],
                        wo_sb[fc][:, og * 512:(og + 1) * 512],
                        start=(fc == 0), stop=(fc == FC - 1),
                    )
                ot = spool.tile([P, 512], f32, tag="osb", name="osb", bufs=2)
                nc.vector.tensor_copy(ot[:], ps[:])
                nc.sync.dma_start(out_t[t][:, og * 512:(og + 1) * 512],
                                  ot[:])

            # ---- attention: fused QK/exp/PV loop ----
            # Per (hp, sg): PV chain links for sk-1 / sk-2 are interleaved
            # into the QK loop (lag 1 and 2), so the PE keeps working while
            # ACT exps the current tile and ACT never waits long for scores.
            # fillers: list of callables issued at fixed sk slots.
            def attn_group(hp, sg, fillers=()):
                at_tiles = []
                cps = [None, None]

                def pv_link(h2, sk):
                    if cps[h2] is None:
                        # lazy: the ring-slot wait lands after the previous
                        # group's normalization has freed its buffer
                        cps[h2] = pvps.tile([DK + 1, 512], f32, tag="pv",
                                            name="ps_ctx")
                    nc.tensor.matmul(
                        cps[h2][:],
                        v_sb[sk][:, hp * 2 + h2, :],
                        at_tiles[sk][:, h2 * 512:(h2 + 1) * 512],
                        start=(sk == 0), stop=(sk == ST - 1),
                    )

                fill_slots = {}
                if fillers:
                    step = max(1, ST // len(fillers))
                    for i, f in enumerate(fillers):
                        fill_slots[min(i * step, ST - 1)] = f
                for sk in range(ST):
                    ps = mmps.tile([P, 1024], f32, tag="mm", name="ps_qk")
                    for h2 in range(2):
                        hq = slice(h2 * DK, (h2 + 1) * DK)
                        nc.tensor.matmul(
                            ps[:, h2 * 512:(h2 + 1) * 512],
                            kt_sb[hp][hq, sk * P:(sk + 1) * P],
                            qt_sb[hp][hq, sg * 512:(sg + 1) * 512],
                            start=True, stop=True,
                        )
                    at = apool.tile([P, 1024], bf16, tag="attn",
                                    name="attn")
                    nc.scalar.activation(at[:], ps[:], EXP, scale=0.125)
                    at_tiles.append(at)
                    if sk in fill_slots:
                        fill_slots[sk]()
                    if sk >= 1:
                        pv_link(0, sk - 1)
                    if sk >= 2:
                        pv_link(1, sk - 2)
                pv_link(0, ST - 1)
                pv_link(1, ST - 2)
                pv_link(1, ST - 1)

                for h2 in range(2):
                    cp = cps[h2]
                    rin = spool.tile([1, 512], f32, tag="rin", name="rin",
                                     bufs=2)
                    nc.vector.reciprocal(rin[0:1, :], cp[DK:DK + 1, :])
                    bcs = spool.tile([DK, 512], f32, tag="bcs", name="bcs")
                    nc.gpsimd.partition_broadcast(bcs[:], rin[0:1, :])
                    if h2 == 0:
                        nc.vector.tensor_mul(
                            ctx_sb[hp][0:DK, sg * 512:(sg + 1) * 512],
                            cp[0:DK, :], bcs[:])
                    else:
                        tmp = spool.tile([DK, 512], bf16, tag="ctmp",
                                         name="ctmp", bufs=2)
                        nc.vector.tensor_mul(tmp[:], cp[0:DK, :], bcs[:])
                        nc.sync.dma_start(
                            ctx_sb[hp][DK:P, sg * 512:(sg + 1) * 512],
                            tmp[:])

            # -- sg0: K/Q/V projections pipelined into the hp groups --
            # hp0: V-proj groups ride inside the QK loop (vp(sk) lands one
            # step before pv_link(0, sk) needs it).
            k_proj_fc(0)
            q_proj_group(0, 0)
            vps = [(lambda t=t: v_proj_group(t)) for t in range(ST)]
            attn_group(0, 0, fillers=vps)
            k_proj_fc(1)
            q_proj_group(0, 1)
            attn_group(1, 0)
            k_proj_fc(2)
            q_proj_group(0, 2)
            attn_group(2, 0)
            k_proj_fc(3)
            q_proj_group(0, 3)
            attn_group(3, 0)

            # -- sg1..3: Q-proj(sg,hp+1) and out-proj(sg-1) in QK loops --
            for sg in range(1, SG):
                if sg >= 2:
                    issue_xq_dma(sg)
                q_proj_group(sg, 0)
                base = (sg - 1) * 4
                for hp in range(FC):
                    fillers = [
                        (lambda t=base + hp, og=0: out_proj_group(t, og)),
                        (lambda t=base + hp, og=1: out_proj_group(t, og)),
                    ]
                    if hp < FC - 1:
                        fillers.insert(
                            0, (lambda sg=sg, fc=hp + 1:
                                q_proj_group(sg, fc)))
                    attn_group(hp, sg, fillers=fillers)

            # tail: out projection for sg=3
            for t in range(12, 16):
                for og in range(2):
                    out_proj_group(t, og)

    nc.compile()
    return nc


_NC_CACHE = None


def _get_program():
    global _NC_CACHE
    if _NC_CACHE is None:
        _NC_CACHE = _build_program()
    return _NC_CACHE


def kernel(q, k, v, W_q, W_k, W_v, W_o):
    from concourse.bass_utils import run_bass_kernel_spmd

    q = np.asarray(q, np.float32)
    k = np.asarray(k, np.float32)
    v = np.asarray(v, np.float32)
    W_q = np.asarray(W_q, np.float32)
    W_k = np.asarray(W_k, np.float32)
    W_v = np.asarray(W_v, np.float32)
    W_o = np.asarray(W_o, np.float32)

    nc = _get_program()
    in_maps = []
    for c in range(NCORES):
        b, g = c // 2, c % 2
        sl = slice(g * F, (g + 1) * F)
        in_maps.append({
            "xq": np.ascontiguousarray(q[b].T).astype(BF),
            "xk": np.ascontiguousarray(k[b].T).astype(BF),
            "xv": np.ascontiguousarray(v[b].T).astype(BF),
            "wq": np.ascontiguousarray(W_q[sl, :].T).astype(BF),
            "wk": np.ascontiguousarray(W_k[sl, :].T).astype(BF),
            "wv": np.ascontiguousarray(W_v[sl, :].T).astype(BF),
            "wo": np.ascontiguousarray(W_o[:, sl].T).astype(BF),
        })
    res = run_bass_kernel_spmd(nc, in_maps, list(range(NCORES)))
    outs = [res.results[c]["out"] for c in range(NCORES)]
    full = np.stack([outs[2 * b] + outs[2 * b + 1] for b in range(4)])
    return full.astype(np.float32)
